# revision 1
# baseline (speedup 1.0000x reference)
"""Multi-head self-attention on 8 Trainium2 NeuronCores.

Problem: x:(4,2048,1024) fp32; q = x@Wq, kv = x@Wkv (k,v split), 8 heads of
dim 64, softmax(q k^T / 8) v, concat heads, @Wo + bo -> (4,2048,1024).

Sharding: core c handles batch b=c//2 and head group g=c%2 (4 of 8 heads).
Each core computes its batch's projections restricted to its 4 heads, full
attention for those heads, and a partial output projection y_c = U_norm @ Wo_g.
Host gathers: out[b] = y_{2b} + y_{2b+1} + bo  (the "all-reduce" of the
tensor-parallel head split, done at unshard time).

Device algorithm (per core), all matmul operands fp16, PSUM accumulate fp32:
  - host supplies xT = x[b].T so the contraction dim (QDIM) is the partition
    axis; projections compute qT/kT (head_dim-major) and v (seq-major) tiles.
  - attention per head, per i-half (1024 q rows), per j-tile (128 k rows):
      simT[j,i] = kT_h(j)^T-tile @ qT_h        (PE, K=64)
      expT = exp(SCALE*simT)                   (ACT, reads PSUM directly)
      U~[d,i] += [v_h | 1]^T @ expT            (PE, K=128; row 64 = softmax sum)
    then normalization: r = 1/s via fast-reciprocal (DVE), R = ones x r
    broadcast (PE K=1 matmul), U_norm = U~ * R (DVE).
  - y[m,:] = U_norm_pairs^T @ Wo_g (K=128 per head pair), DVE drain, DMA out.
"""

import numpy as np

# ---- problem constants (hardcoded per the harness contract) ----
B, N, QDIM = 4, 2048, 1024
HEADS, DIM_MODEL = 8, 512
HEAD_DIM = DIM_MODEL // HEADS  # 64
SCALE = HEAD_DIM ** -0.5  # 0.125
N_CORES = 8
HEADS_PER_CORE = HEADS // 2  # 4 (head-group split across 2 cores per batch)
DMC = HEADS_PER_CORE * HEAD_DIM  # 256 per-core model dim slice


def build_nc(seq=N, qd=QDIM, nh=HEADS_PER_CORE, hd=HEAD_DIM, dout=QDIM,
             scale=SCALE, ihw=1024, skip_norm=False, norm_mode='dve',
             expp_bufs=3, upool_bufs=2, rows_bufs=2, ysb_bufs=3,
             phases='all', simp_bufs=2, uaccp_bufs=1, spare_bufs=2,
             xt_one_dma=True, y_pair_dma=True):
    """Build the per-core Bass program (same program on all 8 cores)."""
    from contextlib import ExitStack

    import concourse.bass as bass
    import concourse.tile as tile
    from concourse import bacc, mybir

    P = 128
    NC5 = 512  # psum bank width in fp32
    f16 = mybir.dt.float16
    f32 = mybir.dt.float32
    Exp = mybir.ActivationFunctionType.Exp
    Ln = mybir.ActivationFunctionType.Ln

    dmc = nh * hd                 # per-core projected dim (256)
    kt = qd // P                  # contraction tiles over QDIM (8)
    seqt = seq // P               # seq tiles (16)
    mtiles = max(1, dmc // P)     # qT/kT partition tiles (2)
    heads_per_mtile = nh // mtiles
    ihw = min(ihw, seq)           # i-half width
    n_ih = seq // ihw
    npairs = mtiles               # head pairs stacked for final proj (2)

    def chunks(total, w=NC5):
        c0 = 0
        while c0 < total:
            yield c0, min(w, total - c0)
            c0 += w

    nc = bacc.Bacc("TRN2", target_bir_lowering=False, debug=False,
                   num_devices=N_CORES)

    xt = nc.dram_tensor("xt", (qd, seq), f16, kind="ExternalInput").ap()
    wq = nc.dram_tensor("wq", (qd, dmc), f16, kind="ExternalInput").ap()
    wk = nc.dram_tensor("wk", (qd, dmc), f16, kind="ExternalInput").ap()
    wv = nc.dram_tensor("wv", (qd, dmc), f16, kind="ExternalInput").ap()
    wo = nc.dram_tensor("wo", (dmc, dout), f16, kind="ExternalInput").ap()
    y = nc.dram_tensor("y", (seq, dout), f32, kind="ExternalOutput").ap()

    with tile.TileContext(nc) as tc, ExitStack() as ctx:
        # ---- SBUF pools ----
        persist = ctx.enter_context(tc.tile_pool(name="persist", bufs=1))
        expp = ctx.enter_context(tc.tile_pool(name="expp", bufs=expp_bufs))
        upool = ctx.enter_context(tc.tile_pool(name="upool", bufs=upool_bufs))
        rows = ctx.enter_context(tc.tile_pool(name="rows", bufs=rows_bufs))
        ysb = ctx.enter_context(tc.tile_pool(name="ysb", bufs=ysb_bufs))
        # ---- PSUM pools (8 banks total: 2 spare + 4 sim + 2 uacc) ----
        spare = ctx.enter_context(tc.tile_pool(name="spare", bufs=spare_bufs, space="PSUM"))
        simp = ctx.enter_context(tc.tile_pool(name="simp", bufs=simp_bufs, space="PSUM"))
        uaccp = ctx.enter_context(tc.tile_pool(name="uaccp", bufs=uaccp_bufs, space="PSUM"))

        # ---- persistent SBUF tensors ----
        xt_sb = persist.tile([P, kt, seq], f16)
        wq_sb = persist.tile([P, kt, dmc], f16)
        wk_sb = persist.tile([P, kt, dmc], f16)
        wv_sb = persist.tile([P, kt, dmc], f16)
        wo_sb = persist.tile([min(P, dmc), npairs, dout], f16)
        v_sb = persist.tile([P, seqt, nh, hd + 1], f16)
        qt_sb = persist.tile([min(P, dmc), mtiles, seq], f16)
        kt_sb = persist.tile([min(P, dmc), mtiles, seq], f16)
        upairs = [persist.tile([min(P, dmc), seq], f16, name=f"upair{p}")
                  for p in range(npairs)]
        ones65 = persist.tile([65, hd], f16)

        # ---- input loads ----
        if xt_one_dma:
            nc.sync.dma_start(xt_sb[:], xt.rearrange("(ko ki) s -> ki ko s",
                                                     ki=P))
        else:
            for ko in range(kt):
                nc.sync.dma_start(xt_sb[:, ko, :], xt[ko * P:(ko + 1) * P, :])
        nc.sync.dma_start(wk_sb[:], wk.rearrange("(ko ki) m -> ki ko m", ki=P))
        nc.sync.dma_start(wq_sb[:], wq.rearrange("(ko ki) m -> ki ko m", ki=P))
        nc.sync.dma_start(wv_sb[:], wv.rearrange("(ko ki) m -> ki ko m", ki=P))
        nc.sync.dma_start(wo_sb[:], wo.rearrange("(t p) n -> p t n", p=min(P, dmc)))
        nc.vector.memset(v_sb[:, :, :, hd:hd + 1], 1.0)
        nc.vector.memset(ones65[:], 1.0)

        def proj_kq_tile(mt, which, n0, nw):
            """One [mp, nw] tile of kT (which=0) or qT (which=1) for m-tile mt."""
            mp = min(P, dmc)
            w_sb, out_sb = ((wk_sb, kt_sb), (wq_sb, qt_sb))[which]
            ps = spare.tile([mp, NC5], f32, tag="ps512", name="ps")
            for ko in range(kt):
                nc.tensor.matmul(
                    ps[:, 0:nw],
                    lhsT=w_sb[:, ko, mt * mp:(mt + 1) * mp],
                    rhs=xt_sb[:, ko, n0:n0 + nw],
                    start=(ko == 0), stop=(ko == kt - 1))
            nc.vector.tensor_copy(
                out_sb[0:mp, mt, n0:n0 + nw], ps[:, 0:nw])

        def proj_v_tile(jt):
            """v natural layout [seq, dmc] -> v_sb[:, jt, h, 0:hd]."""
            ps = spare.tile([P, dmc], f32, tag="ps512", name="ps")
            for ko in range(kt):
                nc.tensor.matmul(
                    ps[:],
                    lhsT=xt_sb[:, ko, jt * P:(jt + 1) * P],
                    rhs=wv_sb[:, ko, :],
                    start=(ko == 0), stop=(ko == kt - 1))
            nc.vector.tensor_copy(
                v_sb[:, jt, :, 0:hd],
                ps.rearrange("p (h d) -> p h d", h=nh))

        def attn_head_ih(h, ih, nm_override=None, pre_norm_cb=None):
            if True:
                nmode = nm_override or norm_mode
                mt = h // heads_per_mtile
                hb = (h % heads_per_mtile) * hd
                pair = h // heads_per_mtile
                i0 = ih * ihw
                uacc = uaccp.tile([hd + 1, ihw], f32, tag="uacc")
                for jt in range(seqt):
                    sim = simp.tile([P, ihw], f32, tag="sim")
                    for c0, cw in chunks(ihw):
                        nc.tensor.matmul(
                            sim[:, c0:c0 + cw],
                            lhsT=kt_sb[hb:hb + hd, mt, jt * P:(jt + 1) * P],
                            rhs=qt_sb[hb:hb + hd, mt, i0 + c0:i0 + c0 + cw],
                            start=True, stop=True)
                    expt = expp.tile([P, ihw], f16, tag="expt")
                    nc.scalar.activation(expt[:], sim[:], Exp, scale=scale)
                    for c0, cw in chunks(ihw):
                        nc.tensor.matmul(
                            uacc[:, c0:c0 + cw],
                            lhsT=v_sb[:, jt, h, :],
                            rhs=expt[:, c0:c0 + cw],
                            start=(jt == 0), stop=(jt == seqt - 1))
                if pre_norm_cb is not None:
                    # emit next phase's projection granules here so their DVE
                    # drains are ordered BEFORE this phase's norm chain
                    pre_norm_cb()
                # normalization: r = 1/s; U_norm = U~ * broadcast(r)
                u_sb = upool.tile([hd, ihw], f16, tag="u")
                nc.vector.tensor_copy(u_sb[:], uacc[0:hd, :])
                if skip_norm:
                    nc.vector.tensor_copy(
                        upairs[pair][hb:hb + hd, i0:i0 + ihw], u_sb[:])
                    return
                srow = rows.tile([65, ihw], f32, tag="srow")
                nc.vector.tensor_copy(srow[64:65, :], uacc[hd:hd + 1, :])
                # r = 1/s. The fused custom-DVE reciprocal op returns garbage
                # on this HW path, so either ACT ln/exp ("ln") or a manual
                # Newton iteration from standard DVE ops ("dve", default —
                # keeps the critical ACT engine free for the softmax exps).
                rrow = rows.tile([65, ihw], f32, tag="rrow")
                rrow16 = rows.tile([65, ihw], f16, tag="rrow16")
                if nmode == "ln":
                    lnrow = rows.tile([65, ihw], f32, tag="lnrow")
                    nc.scalar.activation(lnrow[64:65, :], srow[64:65, :], Ln)
                    nc.scalar.activation(rrow[64:65, :], lnrow[64:65, :], Exp,
                                         scale=-1.0)
                    nc.vector.tensor_copy(rrow16[64:65, :], rrow[64:65, :])
                elif nmode == "dve":
                    i32 = mybir.dt.int32
                    s_r, u_r, t_r = (srow[64:65, :], rrow[64:65, :],
                                     None)
                    trow = rows.tile([65, ihw], f32, tag="trow")
                    t_r = trow[64:65, :]
                    # u0 = bitcast(~bits(s)) * 0.23549792   (u = -1/s approx)
                    nc.vector.tensor_scalar(t_r.bitcast(i32), s_r.bitcast(i32),
                                            -1, None,
                                            op0=mybir.AluOpType.bitwise_xor)
                    nc.vector.tensor_scalar_mul(u_r, t_r, 0.23549792)
                    # two Newton passes: u <- (s*u + c)*u, c = 2.0017324, 2.0
                    for c in (2.0017324, 2.0):
                        nc.vector.tensor_mul(t_r, s_r, u_r)
                        nc.vector.scalar_tensor_tensor(
                            u_r, t_r, float(c), u_r,
                            op0=mybir.AluOpType.add, op1=mybir.AluOpType.mult)
                    # r = -u, cast to fp16
                    nc.vector.tensor_scalar_mul(rrow16[64:65, :], u_r, -1.0)
                elif nmode == "copy":  # timing-only bisect: wrong math
                    nc.vector.tensor_copy(rrow16[64:65, :], srow[64:65, :])
                else:
                    raise ValueError(nmode)
                for c0, cw in chunks(ihw):
                    rps = spare.tile([hd, NC5], f32, tag="ps512")
                    nc.tensor.matmul(
                        rps[:, 0:cw],
                        lhsT=ones65[64:65, :],
                        rhs=rrow16[64:65, c0:c0 + cw],
                        start=True, stop=True)
                    nc.vector.tensor_mul(
                        upairs[pair][hb:hb + hd, i0 + c0:i0 + c0 + cw],
                        u_sb[:, c0:c0 + cw], rps[:, 0:cw])

        def final_proj(ms=None):
            mp = min(P, dmc)
            for m in (range(seqt) if ms is None else ms):
                if y_pair_dma:
                    yt = ysb.tile([P, dout], f32, tag="yt")
                for n0, nw in chunks(dout):
                    yps = spare.tile([P, NC5], f32, tag="ps512")
                    for p in range(npairs):
                        nc.tensor.matmul(
                            yps[:, 0:nw],
                            lhsT=upairs[p][0:mp, m * P:(m + 1) * P],
                            rhs=wo_sb[0:mp, p, n0:n0 + nw],
                            start=(p == 0), stop=(p == npairs - 1))
                    if y_pair_dma:
                        nc.vector.tensor_copy(yt[:, n0:n0 + nw], yps[:, 0:nw])
                    else:
                        yt = ysb.tile([P, NC5], f32, tag="yt")
                        nc.vector.tensor_copy(yt[:, 0:nw], yps[:, 0:nw])
                        nc.sync.dma_start(
                            y[m * P:(m + 1) * P, n0:n0 + nw], yt[:, 0:nw])
                if y_pair_dma:
                    nc.sync.dma_start(y[m * P:(m + 1) * P, :], yt[:])

        # Emission schedule: per-engine instruction order is static after
        # scheduling, so projection granules are threaded between attention
        # (h, ih) phases — each phase's inputs emitted one phase ahead; the
        # ACT-paced attention then hides the remaining projection PE work.
        attn_phases = [(h, ih) for h in range(nh) for ih in range(n_ih)]

        def phase_needs(idx):
            # granules that must be emitted before attention phase idx;
            # every phase's j-loop consumes ALL v tiles, so v has deadline 0.
            if idx >= len(attn_phases):
                return []
            h, ih = attn_phases[idx]
            mt = h // heads_per_mtile
            need = [("k", mt, n0, nw) for n0, nw in chunks(seq)]
            need += [("q", mt, n0, nw) for n0, nw in chunks(seq)
                     if n0 < (ih + 1) * ihw and n0 + nw > ih * ihw]
            if idx == 0:
                need += [("v", jt) for jt in range(seqt)]
            return need

        emitted = set()

        def emit_granules(needs):
            for g in needs:
                if g in emitted:
                    continue
                emitted.add(g)
                if g[0] == "v":
                    proj_v_tile(g[1])
                else:
                    which = 0 if g[0] == "k" else 1
                    proj_kq_tile(g[1], which, g[2], g[3])

        all_granules = []
        for idx in range(len(attn_phases)):
            for g in phase_needs(idx):
                if g not in all_granules:
                    all_granules.append(g)

        if phases == 'proj':
            emit_granules(all_granules)
        else:
            emit_granules(phase_needs(0))
            # deadline-ordered backlog, spread evenly across early boundaries
            backlog = [g for g in all_granules if g not in emitted]
            nb = max(1, len(attn_phases) - 2)
            share = -(-len(backlog) // nb)
            last = len(attn_phases) - 1
            for idx, (h, ih) in enumerate(attn_phases):
                def _cb(idx=idx):
                    emit_granules(phase_needs(idx + 1))
                    take = [g for g in backlog if g not in emitted][:share]
                    emit_granules(take)
                attn_head_ih(h, ih, nm_override="ln" if idx == last else None,
                             pre_norm_cb=_cb)
                if phases == 'all' and idx == last - 1 and n_ih > 1:
                    # final-proj m-tiles whose i-range completes at the
                    # second-to-last phase overlap the last phase's attention
                    lh, lih = attn_phases[last]
                    done_ih = [p_ih for p_ih in range(n_ih) if p_ih != lih]
                    ms = [m for m in range(seqt)
                          if (m * P) // ihw in done_ih]
                    final_proj(ms)
            if phases == 'all':
                lh, lih = attn_phases[last]
                if n_ih > 1:
                    final_proj([m for m in range(seqt)
                                if (m * P) // ihw == lih])
                else:
                    final_proj()

    nc.compile()
    return nc


_NC_CACHE = {}


def _get_nc():
    if "nc" not in _NC_CACHE:
        _NC_CACHE["nc"] = build_nc()
    return _NC_CACHE["nc"]


def _prep_core_inputs(x, Wq, Wkv, Wo):
    """Host-side shard + layout prep: per-core fp16 slices."""
    f16 = np.float16
    in_maps = []
    for c in range(N_CORES):
        b, g = c // 2, c % 2
        s = slice(g * DMC, (g + 1) * DMC)
        in_maps.append({
            "xt": np.ascontiguousarray(x[b].T).astype(f16),
            "wq": np.ascontiguousarray(Wq[:, s]).astype(f16),
            "wk": np.ascontiguousarray(Wkv[:, g * DMC:(g + 1) * DMC]).astype(f16),
            "wv": np.ascontiguousarray(
                Wkv[:, DIM_MODEL + g * DMC:DIM_MODEL + (g + 1) * DMC]).astype(f16),
            "wo": np.ascontiguousarray(Wo[s, :]).astype(f16),
        })
    return in_maps


def kernel(x, Wq, Wkv, Wo, bo):
    from concourse import bass_utils

    x = np.asarray(x, dtype=np.float32)
    Wq = np.asarray(Wq, dtype=np.float32)
    Wkv = np.asarray(Wkv, dtype=np.float32)
    Wo = np.asarray(Wo, dtype=np.float32)
    bo = np.asarray(bo, dtype=np.float32)

    nc = _get_nc()
    in_maps = _prep_core_inputs(x, Wq, Wkv, Wo)
    res = bass_utils.run_bass_kernel_spmd(nc, in_maps,
                                          core_ids=list(range(N_CORES)))
    out = np.empty((B, N, QDIM), dtype=np.float32)
    for b in range(B):
        out[b] = res.results[2 * b]["y"] + res.results[2 * b + 1]["y"] + bo
    return out



# revision 50
# speedup vs baseline: 1.3452x; 1.3452x over previous
"""Multi-head self-attention on 8 Trainium2 NeuronCores.

Problem: x:(4,2048,1024) fp32; q = x@Wq, kv = x@Wkv (k,v split), 8 heads of
dim 64, softmax(q k^T / 8) v, concat heads, @Wo + bo -> (4,2048,1024).

Sharding: core c handles batch b=c//2 and head group g=c%2 (4 of 8 heads).
Each core computes its batch's projections restricted to its 4 heads, full
attention for those heads, and a partial output projection y_c = U_norm @ Wo_g.
Host gathers: out[b] = y_{2b} + y_{2b+1} + bo  (the "all-reduce" of the
tensor-parallel head split, done at unshard time).

Device algorithm (per core), matmul operands fp16, PSUM accumulate fp32:
  - projections qT/kT (head_dim-major) and v (seq-major), contraction over
    QDIM on partitions, emitted as micro-granules threaded into PE gaps.
  - attention per head, per i-half (1024 q rows):
      simT[j,i] = kT_h(j)-tile^T @ qT_h          (PE, K=64)
      expT = exp(SCALE*simT)                     (ACT, reads PSUM; a few
        j-tiles instead use a Schraudolph bit-trick exp on the idle Pool
        engine: i16 = a*sim + b, bitcast fp16)
      per i-subtile s (128 rows): uacc[s][i,0:65] += expT_s^T @ [v_h | 1]
        (PE, streams only 65 rows/j-tile; col 64 = softmax sum)
    normalization: r = 1/s per (i) via Newton bit-trick (DVE) or ln/exp
    (ACT, last phase), U_norm = uacc * r (DVE per-partition scalar),
    then PE-transpose (identity matmul) into upairs[dm, i] fp16.
  - y[m,:] = upairs^T @ Wo (K=128 per pair), fp16 DMA out; host adds halves.
"""

import numpy as np

# ---- problem constants (hardcoded per the harness contract) ----
B, N, QDIM = 4, 2048, 1024
HEADS, DIM_MODEL = 8, 512
HEAD_DIM = DIM_MODEL // HEADS  # 64
SCALE = HEAD_DIM ** -0.5  # 0.125
N_CORES = 8
HEADS_PER_CORE = HEADS // 2  # 4
DMC = HEADS_PER_CORE * HEAD_DIM  # 256 per-core model dim slice

# Schraudolph fp16 exp: i16 = rint(a*x + b); bitcast f16 ~= exp(SCALE*x)
SCH_A = SCALE * 1024.0 / np.log(2.0)  # 184.6650
SCH_B = 15360.0 - 59.0


def build_nc(seq=N, qd=QDIM, nh=HEADS_PER_CORE, hd=HEAD_DIM, dout=QDIM,
             scale=SCALE, pool_jts=(), dve_jts=(3, 5, 8, 10, 13, 15),
             dve_jts_late=None, late_from=8,
             pool_skip_first=1, last_norm='dve', norm_mode='dve',
             gap_ns=390, expp_bufs=8, y16=True, ihw=512, h_outer=False,
             norm_eng='pool', pend_jt=4, ts_eng='dve', kq_act=1,
             first_split=False, last_split=False, kq1_after=7, up_act=0,
             budget_f=1.15,
             sim_bufs=4, uacc_bufs=4, dbg=False, eager=False):
    """Build the per-core Bass program (same program on all 8 cores)."""
    from contextlib import ExitStack

    import concourse.bass as bass
    import concourse.tile as tile
    from concourse import bacc, mybir

    P = 128
    f16 = mybir.dt.float16
    f32 = mybir.dt.float32
    i16 = mybir.dt.int16
    i32 = mybir.dt.int32
    Exp = mybir.ActivationFunctionType.Exp
    Ln = mybir.ActivationFunctionType.Ln
    Mul = mybir.AluOpType.mult
    Add = mybir.AluOpType.add
    Xor = mybir.AluOpType.bitwise_xor

    dmc = nh * hd                  # 256
    kt = qd // P                   # 8 contraction tiles
    seqt = seq // P                # 16 j-tiles
    mtiles = dmc // P              # 2 qT/kT partition tiles
    npairs = mtiles
    n_ih = seq // ihw              # 2 i-halves per head
    nsub = ihw // P                # 8 i-subtiles per phase
    ncw = seq // 512               # 4 column chunks (proj granule width 512)
    ydt = f16 if y16 else f32

    nc = bacc.Bacc("TRN2", target_bir_lowering=False, debug=False,
                   num_devices=N_CORES)

    xt = nc.dram_tensor("xt", (qd, seq), f16, kind="ExternalInput").ap()
    wq = nc.dram_tensor("wq", (qd, dmc), f16, kind="ExternalInput").ap()
    wk = nc.dram_tensor("wk", (qd, dmc), f16, kind="ExternalInput").ap()
    wv = nc.dram_tensor("wv", (qd, dmc), f16, kind="ExternalInput").ap()
    wo = nc.dram_tensor("wo", (dmc, dout), f16, kind="ExternalInput").ap()
    ident = nc.dram_tensor("ident", (P, P), f16, kind="ExternalInput").ap()
    y = nc.dram_tensor("y", (seq, dout), ydt, kind="ExternalOutput").ap()
    dbgu = (nc.dram_tensor("dbgu", (P, npairs, seq), f16,
                           kind="ExternalOutput").ap() if dbg else None)
    dbgq = (nc.dram_tensor("dbgq", (P, mtiles, seq), f16,
                           kind="ExternalOutput").ap() if dbg else None)
    dbgk = (nc.dram_tensor("dbgk", (P, mtiles, seq), f16,
                           kind="ExternalOutput").ap() if dbg else None)
    dbgv = (nc.dram_tensor("dbgv", (P, seqt, nh, hd + 1), f16,
                           kind="ExternalOutput").ap() if dbg else None)
    dbgs = (nc.dram_tensor("dbgs", (P, 512), f32,
                           kind="ExternalOutput").ap() if dbg else None)
    dbga = (nc.dram_tensor("dbga", (P, 4, P), f32,
                           kind="ExternalOutput").ap() if dbg else None)
    dbge = (nc.dram_tensor("dbge", (P, 512), f16,
                           kind="ExternalOutput").ap() if dbg else None)

    with tile.TileContext(nc) as tc, ExitStack() as ctx:
        # ---- SBUF pools ----
        persist = ctx.enter_context(tc.tile_pool(name="persist", bufs=1))
        expp = ctx.enter_context(tc.tile_pool(name="expp", bufs=expp_bufs))
        u16p = ctx.enter_context(tc.tile_pool(name="u16p", bufs=2))
        rws = ctx.enter_context(tc.tile_pool(name="rws", bufs=2))
        ysb = ctx.enter_context(tc.tile_pool(name="ysb", bufs=3))
        # ---- PSUM pools (8 banks: 4 sim + 2 uacc + 2 spare) ----
        simp = ctx.enter_context(tc.tile_pool(name="simp", bufs=sim_bufs,
                                              space="PSUM"))
        uaccp = ctx.enter_context(tc.tile_pool(name="uaccp", bufs=uacc_bufs,
                                               space="PSUM"))
        spare = simp

        # ---- persistent SBUF tensors ----
        xt_sb = persist.tile([P, kt, seq], f16)
        wq_sb = persist.tile([P, kt, dmc], f16)
        wk_sb = persist.tile([P, kt, dmc], f16)
        wv_sb = persist.tile([P, kt, dmc], f16)
        wo_sb = persist.tile([P, npairs, dout], f16)
        v_sb = persist.tile([P, seqt, nh, hd + 1], f16)
        qt_sb = persist.tile([P, mtiles, seq], f16)
        kt_sb = persist.tile([P, mtiles, seq], f16)
        ident_sb = persist.tile([P, P], f16)
        upairs = [persist.tile([P, seq], f16, name=f"upair{p}")
                  for p in range(npairs)]

        # ---- input DMAs, ordered for earliest compute start ----
        def dma_w(dst, src, lo, hi):
            nc.sync.dma_start(
                dst[:, :, lo:hi],
                src[:, lo:hi].rearrange("(ko ki) m -> ki ko m", ki=P))

        def dma_xt(c):
            nc.sync.dma_start(
                xt_sb[:, :, c * 512:(c + 1) * 512],
                xt[:, c * 512:(c + 1) * 512].rearrange(
                    "(ko ki) s -> ki ko s", ki=P))

        dma_w(wk_sb, wk, 0, P)
        nc.sync.dma_start(
            xt_sb[:, :, 0:256],
            xt[:, 0:256].rearrange("(ko ki) s -> ki ko s", ki=P))
        dma_w(wq_sb, wq, 0, P)
        nc.sync.dma_start(
            xt_sb[:, :, 256:512],
            xt[:, 256:512].rearrange("(ko ki) s -> ki ko s", ki=P))
        dma_xt(1)
        dma_w(wv_sb, wv, 0, dmc)
        dma_xt(2)
        dma_xt(3)
        dma_w(wk_sb, wk, P, dmc)
        dma_w(wq_sb, wq, P, dmc)
        nc.sync.dma_start(wo_sb[:],
                          wo.rearrange("(t p) n -> p t n", p=P))
        nc.sync.dma_start(ident_sb[:], ident[:])
        nc.vector.memset(v_sb[:, :, :, hd], 1.0)

        # ================= micro-task filler =================
        class Gran:
            def __init__(self, key, micros):
                self.key = key
                self.micros = micros  # list of (ns, fn)
                self.i = 0

            def pending(self):
                return self.i < len(self.micros)

            def step(self):
                ns, fn = self.micros[self.i]
                self.i += 1
                fn()
                return ns

        grans = {}
        order = []
        pending_ns = [0.0]

        def add_gran(key, micros):
            g = Gran(key, micros)
            grans[key] = g
            order.append(g)
            pending_ns[0] += sum(ns for ns, _ in micros)

        def force(key):
            g = grans[key]
            while g.pending():
                pending_ns[0] -= g.micros[g.i][0]
                g.step()

        def fill(budget):
            for g in order:
                while g.pending() and budget > 100:
                    ns = g.micros[g.i][0]
                    pending_ns[0] -= ns
                    budget -= ns
                    g.step()
                if budget <= 100:
                    break

        # ---- projection granules ----
        def kq_gran(which, mt, c, n0=None, nw=512):
            """kT (which=0) / qT (which=1) tile [mt*128, n0:n0+nw]."""
            w_sb, out_sb = ((wk_sb, kt_sb), (wq_sb, qt_sb))[which]
            if n0 is None:
                n0 = c * 512
            cell = {}

            def mm(ko):
                def f():
                    if ko == 0:
                        cell["ps"] = spare.tile([P, nw], f32, tag="sim",
                                                name="ps",
                                                padded_shape=[P, 512])
                    for k in (ko, ko + 1):
                        nc.tensor.matmul(
                            cell["ps"][:],
                            lhsT=w_sb[:, k, mt * P:(mt + 1) * P],
                            rhs=xt_sb[:, k, n0:n0 + nw],
                            start=(k == 0), stop=(k == kt - 1))
                    if ko == kt - 2:
                        if kq_act:
                            nc.scalar.activation(
                                out_sb[:, mt, n0:n0 + nw],
                                cell["ps"][:],
                                mybir.ActivationFunctionType.Copy)
                        else:
                            nc.vector.tensor_copy(
                                out_sb[:, mt, n0:n0 + nw],
                                cell["ps"][:])
                return f

            return [(440 * nw // 512 + 30, mm(ko)) for ko in range(0, kt, 2)]

        def v_gran(jt):
            """v_sb[:, jt, :, 0:hd] (2 micros)."""
            cell = {}

            def mm(ko):
                def f():
                    if ko == 0:
                        cell["ps"] = spare.tile([P, dmc], f32, tag="sim",
                                                name="psv")
                    for k in range(ko, ko + 4):
                        nc.tensor.matmul(
                            cell["ps"][:],
                            lhsT=xt_sb[:, k, jt * P:(jt + 1) * P],
                            rhs=wv_sb[:, k, :],
                            start=(k == 0), stop=(k == kt - 1))
                    if ko == kt - 4:
                        nc.vector.tensor_copy(
                            v_sb[:, jt, :, 0:hd],
                            cell["ps"].rearrange("p (h d) -> p h d", h=nh))
                return f

            return [(440, mm(0)), (440, mm(4))]

        def f_gran2(m, act_copy=False):
            """final proj for rows m*128..(m+2)*128: one DMA (6 micros)."""
            cell = {}
            Copy = mybir.ActivationFunctionType.Copy

            def ycopy(dst, src, on_act):
                if on_act:
                    nc.scalar.activation(dst, src, Copy)
                else:
                    nc.vector.tensor_copy(dst, src)

            def mm(t, ci):
                def f():
                    if t == 0 and ci == 0:
                        cell["yt"] = ysb.tile([P, 2, dout], ydt, tag="yt2",
                                              name="yt")
                    if not (t == 0 and ci == 0):
                        pt, pc = (t, ci - 1) if ci else (t - 1, 1)
                        ycopy(cell["yt"][:, pt, pc * 512:(pc + 1) * 512],
                              cell["ps"], (ci == 1) != act_copy)
                    ps = spare.tile([P, 512], f32, tag="sim", name="yps")
                    cell["ps"] = ps
                    for p in range(npairs):
                        nc.tensor.matmul(
                            ps[:],
                            lhsT=upairs[p][:, (m + t) * P:(m + t + 1) * P],
                            rhs=wo_sb[:, p, ci * 512:(ci + 1) * 512],
                            start=(p == 0), stop=(p == npairs - 1))
                return f

            def out():
                ycopy(cell["yt"][:, 1, 512:1024], cell["ps"], not act_copy)
                nc.sync.dma_start(
                    y[m * P:(m + 2) * P, :].rearrange(
                        "(t p) n -> p t n", p=P),
                    cell["yt"][:])

            return ([(470, mm(t, ci)) for t in (0, 1) for ci in (0, 1)]
                    + [(80, out)])

        def f_gran(m, act_copy=False):
            """final proj y[m*128:(m+1)*128, :] (3 micros)."""
            cell = {}
            Copy = mybir.ActivationFunctionType.Copy

            def ycopy(dst, src, on_act):
                if on_act:
                    nc.scalar.activation(dst, src, Copy)
                else:
                    nc.vector.tensor_copy(dst, src)

            def mm(ci):
                def f():
                    if ci == 0:
                        cell["yt"] = ysb.tile([P, dout], ydt, tag="yt",
                                              name="yt")
                    else:
                        ycopy(cell["yt"][:, 0:512], cell["ps0"][:], True)
                        if not act_copy:
                            nc.sync.dma_start(y[m * P:(m + 1) * P, 0:512],
                                              cell["yt"][:, 0:512])
                    ps = spare.tile([P, 512], f32, tag="sim", name="yps")
                    cell[f"ps{ci}"] = ps
                    for p in range(npairs):
                        nc.tensor.matmul(
                            ps[:],
                            lhsT=upairs[p][:, m * P:(m + 1) * P],
                            rhs=wo_sb[:, p, ci * 512:(ci + 1) * 512],
                            start=(p == 0), stop=(p == npairs - 1))
                return f

            def out():
                ycopy(cell["yt"][:, 512:1024], cell["ps1"][:], False)
                if act_copy:
                    nc.sync.dma_start(y[m * P:(m + 1) * P, :], cell["yt"][:])
                else:
                    nc.sync.dma_start(y[m * P:(m + 1) * P, 512:1024],
                                      cell["yt"][:, 512:1024])

            return [(440, mm(0)), (440, mm(1)), (80, out)]

        # deadline-ordered granule queue (finals appended as they unlock)
        for c in range(ncw):
            if c == 0:
                halves = (kq_gran(0, 0, 0, 0, 256)
                          + kq_gran(0, 0, 0, 256, 256))
                add_gran(("k", 0, 0), halves)
                add_gran(("q", 0, 0), (kq_gran(1, 0, 0, 0, 256)
                                       + kq_gran(1, 0, 0, 256, 256)))
                for cq in range(1, ncw):
                    add_gran(("q", 0, cq), kq_gran(1, 0, cq))
            else:
                add_gran(("k", 0, c), kq_gran(0, 0, c))
        for jt in range(seqt):
            add_gran(("v", jt), v_gran(jt))
            if jt == min(kq1_after, seqt - 1):
                add_gran(("k", 1, 0), kq_gran(0, 1, 0))
                add_gran(("q", 1, 0), kq_gran(1, 1, 0))
        for c in range(1, ncw):
            add_gran(("k", 1, c), kq_gran(0, 1, c))
        for c in range(1, ncw):
            add_gran(("q", 1, c), kq_gran(1, 1, c))

        if eager:
            for g in list(order):
                while g.pending():
                    pending_ns[0] -= g.micros[g.i][0]
                    g.step()

        # ================= attention =================
        phases = []
        for q in range(n_ih):
            for h in range(nh):
                i0, iw = q * ihw, ihw
                if q == 0 and h == 0 and first_split:
                    phases += [(h, i0, iw // 2), (h, i0 + iw // 2, iw // 2)]
                elif q == n_ih - 1 and h == nh - 1 and last_split:
                    phases += [(h, i0, iw // 2), (h, i0 + iw // 2, iw // 2)]
                else:
                    phases.append((h, i0, iw))
        # deferred per-phase PE tail (transposes), run early in the NEXT phase
        pending_pe = [None]

        def attn_phase(idx, h, i0, iw):
            mt, hb, pair = h // 2, (h % 2) * hd, h // 2
            nsub = iw // P
            is_last = idx == len(phases) - 1
            nmode = last_norm if is_last else norm_mode
            slots_left = (len(phases) - idx) * seqt
            slot_budget = min(900.0, budget_f * pending_ns[0] / slots_left)
            uacc = [uaccp.tile([P, hd + 1], f32, tag="uacc",
                               name=f"ua{idx}_{s}") for s in range(nsub)]
            sims, expts = {}, {}

            def emit_sim(jt):
                force(("k", mt, (jt * P) // 512))
                for c in range(i0 // 512, (i0 + iw + 511) // 512):
                    force(("q", mt, c))
                sim = simp.tile([P, iw], f32, tag="sim", padded_shape=[P, 512])
                for c0 in range(0, iw, 512):
                    cw = min(512, iw - c0)
                    nc.tensor.matmul(
                        sim[:, c0:c0 + cw],
                        lhsT=kt_sb[hb:hb + hd, mt, jt * P:(jt + 1) * P],
                        rhs=qt_sb[hb:hb + hd, mt, i0 + c0:i0 + c0 + cw],
                        start=True, stop=True)
                sims[jt] = sim

            def emit_exp(jt):
                expt = expp.tile([P, iw], f16, tag="expt",
                                 padded_shape=[P, 512])
                djts = (dve_jts_late if (dve_jts_late is not None
                                         and idx >= late_from) else dve_jts)
                sch = idx >= pool_skip_first and (jt in pool_jts
                                                  or jt in djts)
                if sch:
                    nc.vector.tensor_scalar(
                        expt[:].bitcast(i16), sims[jt][:],
                        float(SCH_A), float(SCH_B), op0=Mul, op1=Add)
                else:
                    nc.scalar.activation(expt[:], sims[jt][:], Exp,
                                         scale=scale)
                expts[jt] = expt

            def emit_av(jt):
                force(("v", jt))
                for s in range(nsub):
                    nc.tensor.matmul(
                        uacc[s][:, 0:hd + 1],
                        lhsT=expts[jt][:, s * P:(s + 1) * P],
                        rhs=v_sb[:, jt, h, :],
                        start=(jt == 0), stop=(jt == seqt - 1))

            depth = sim_bufs - 1
            for j0 in range(depth):
                emit_sim(j0)
                emit_exp(j0)
            for jt in range(seqt + 1):
                fill(slot_budget)
                if jt == pend_jt and pending_pe[0] is not None:
                    pending_pe[0]()
                    pending_pe[0] = None
                if jt + depth < seqt:
                    emit_sim(jt + depth)
                    emit_exp(jt + depth)
                if jt >= 1:
                    emit_av(jt - 1)

            # norm (DVE work, emitted now); transposes deferred to next phase
            u16 = u16p.tile([P, nsub, hd], f16, tag="u16",
                            padded_shape=[P, ihw // P, hd])
            srow = rws.tile([P, nsub], f32, tag="srow", name="srow")
            for s in range(nsub):
                nc.vector.tensor_copy(srow[:, s:s + 1], uacc[s][:, hd:hd + 1])
            rrow = rws.tile([P, nsub], f32, tag="rrow", name="rrow")
            neg = nmode != "ln"
            if nmode == "ln":
                lnrow = rws.tile([P, nsub], f32, tag="lnrow", name="lnrow")
                nc.scalar.activation(lnrow[:], srow[:], Ln)
                nc.scalar.activation(rrow[:], lnrow[:], Exp, scale=-1.0)
            else:
                # Newton bit-trick: u ~= -1/s, two iterations, then negate.
                trow = rws.tile([P, nsub], f32, tag="trow", name="trow")
                if norm_eng == 'pool' and not is_last:
                    ne = nc.gpsimd
                    nc.vector.tensor_scalar(trow[:].bitcast(i32),
                                            srow[:].bitcast(i32), -1, None,
                                            op0=Xor)
                    ne.tensor_scalar_mul(rrow[:], trow[:], 0.23549792)
                    for cst in (2.0017324, 2.0):
                        ne.tensor_mul(trow[:], srow[:], rrow[:])
                        ne.tensor_scalar(trow[:], trow[:], float(cst), None,
                                         op0=Add)
                        ne.tensor_mul(rrow[:], trow[:], rrow[:])
                    ne.tensor_scalar_mul(rrow[:], rrow[:], -1.0)
                else:
                    ne = nc.vector
                    ne.tensor_scalar(trow[:].bitcast(i32),
                                     srow[:].bitcast(i32), -1, None, op0=Xor)
                    ne.tensor_scalar_mul(rrow[:], trow[:], 0.23549792)
                    for cst in (2.0017324, 2.0):
                        ne.tensor_mul(trow[:], srow[:], rrow[:])
                        ne.scalar_tensor_tensor(rrow[:], trow[:], float(cst),
                                                rrow[:], op0=Add, op1=Mul)
                    ne.tensor_scalar_mul(rrow[:], rrow[:], -1.0)
            for s in range(nsub):
                if ts_eng == 'act' and not is_last:
                    nc.scalar.activation(
                        u16[:, s, :], uacc[s][:, 0:hd],
                        mybir.ActivationFunctionType.Copy,
                        scale=rrow[:, s:s + 1])
                else:
                    nc.vector.tensor_scalar(
                        u16[:, s, :], uacc[s][:, 0:hd],
                        rrow[:, s:s + 1], None, op0=Mul)

            def tail():
                for sp in range(nsub // 2):
                    tp = spare.tile([hd, 2 * P], f16, tag="sim", name="tp")
                    nc.tensor.transpose(tp[:, 0:P], u16[:, 2 * sp, :],
                                        ident_sb[:])
                    nc.tensor.transpose(tp[:, P:2 * P], u16[:, 2 * sp + 1, :],
                                        ident_sb[:])
                    udst = upairs[pair][hb:hb + hd,
                                        i0 + sp * 2 * P:i0 + (sp + 1) * 2 * P]
                    if is_last or (up_act and sp % 2 == 0):
                        nc.scalar.activation(
                            udst, tp[:], mybir.ActivationFunctionType.Copy)
                    else:
                        nc.vector.tensor_copy(udst, tp[:])
                if h == nh - 1:
                    for m in range(i0 // P, (i0 + iw) // P, 2):
                        add_gran(("f", m), f_gran2(m, act_copy=is_last))

            pending_pe[0] = tail

        for idx, (h, i0, iw) in enumerate(phases):
            attn_phase(idx, h, i0, iw)
        pending_pe[0]()
        for g in order:
            while g.pending():
                g.step()
        if dbg:
            for p in range(npairs):
                nc.sync.dma_start(dbgu[:, p, :], upairs[p][:])
            nc.sync.dma_start(dbgq[:], qt_sb[:])
            nc.sync.dma_start(dbgk[:], kt_sb[:])
            nc.sync.dma_start(dbgv[:], v_sb[:])

    nc.compile()
    return nc


_NC_CACHE = {}


def _get_nc():
    if "nc" not in _NC_CACHE:
        _NC_CACHE["nc"] = build_nc()
    return _NC_CACHE["nc"]


def _prep_core_inputs(x, Wq, Wkv, Wo):
    """Host-side shard + layout prep: per-core fp16 slices."""
    f16 = np.float16
    eye = np.eye(128, dtype=f16)
    in_maps = []
    for c in range(N_CORES):
        b, g = c // 2, c % 2
        s = slice(g * DMC, (g + 1) * DMC)
        in_maps.append({
            "xt": np.ascontiguousarray(x[b].T).astype(f16),
            "wq": np.ascontiguousarray(Wq[:, s]).astype(f16),
            "wk": np.ascontiguousarray(Wkv[:, g * DMC:(g + 1) * DMC]).astype(f16),
            "wv": np.ascontiguousarray(
                Wkv[:, DIM_MODEL + g * DMC:DIM_MODEL + (g + 1) * DMC]).astype(f16),
            "wo": np.ascontiguousarray(Wo[s, :]).astype(f16),
            "ident": eye,
        })
    return in_maps


def kernel(x, Wq, Wkv, Wo, bo):
    from concourse import bass_utils

    x = np.asarray(x, dtype=np.float32)
    Wq = np.asarray(Wq, dtype=np.float32)
    Wkv = np.asarray(Wkv, dtype=np.float32)
    Wo = np.asarray(Wo, dtype=np.float32)
    bo = np.asarray(bo, dtype=np.float32)

    nc = _get_nc()
    in_maps = _prep_core_inputs(x, Wq, Wkv, Wo)
    res = bass_utils.run_bass_kernel_spmd(nc, in_maps,
                                          core_ids=list(range(N_CORES)))
    out = np.empty((B, N, QDIM), dtype=np.float32)
    for b in range(B):
        out[b] = (res.results[2 * b]["y"].astype(np.float32)
                  + res.results[2 * b + 1]["y"].astype(np.float32) + bo)
    return out


# revision 52
# speedup vs baseline: 1.3470x; 1.0013x over previous
"""Multi-head self-attention on 8 Trainium2 NeuronCores.

Problem: x:(4,2048,1024) fp32; q = x@Wq, kv = x@Wkv (k,v split), 8 heads of
dim 64, softmax(q k^T / 8) v, concat heads, @Wo + bo -> (4,2048,1024).

Sharding: core c handles batch b=c//2 and head group g=c%2 (4 of 8 heads).
Each core computes its batch's projections restricted to its 4 heads, full
attention for those heads, and a partial output projection y_c = U_norm @ Wo_g.
Host gathers: out[b] = y_{2b} + y_{2b+1} + bo  (the "all-reduce" of the
tensor-parallel head split, done at unshard time).

Device algorithm (per core), matmul operands fp16, PSUM accumulate fp32:
  - projections qT/kT (head_dim-major) and v (seq-major), contraction over
    QDIM on partitions, emitted as micro-granules threaded into PE gaps.
  - attention per head, per i-half (1024 q rows):
      simT[j,i] = kT_h(j)-tile^T @ qT_h          (PE, K=64)
      expT = exp(SCALE*simT)                     (ACT, reads PSUM; a few
        j-tiles instead use a Schraudolph bit-trick exp on the idle Pool
        engine: i16 = a*sim + b, bitcast fp16)
      per i-subtile s (128 rows): uacc[s][i,0:65] += expT_s^T @ [v_h | 1]
        (PE, streams only 65 rows/j-tile; col 64 = softmax sum)
    normalization: r = 1/s per (i) via Newton bit-trick (DVE) or ln/exp
    (ACT, last phase), U_norm = uacc * r (DVE per-partition scalar),
    then PE-transpose (identity matmul) into upairs[dm, i] fp16.
  - y[m,:] = upairs^T @ Wo (K=128 per pair), fp16 DMA out; host adds halves.
"""

import numpy as np

# ---- problem constants (hardcoded per the harness contract) ----
B, N, QDIM = 4, 2048, 1024
HEADS, DIM_MODEL = 8, 512
HEAD_DIM = DIM_MODEL // HEADS  # 64
SCALE = HEAD_DIM ** -0.5  # 0.125
N_CORES = 8
HEADS_PER_CORE = HEADS // 2  # 4
DMC = HEADS_PER_CORE * HEAD_DIM  # 256 per-core model dim slice

# Schraudolph fp16 exp: i16 = rint(a*x + b); bitcast f16 ~= exp(SCALE*x)
SCH_A = SCALE * 1024.0 / np.log(2.0)  # 184.6650
SCH_B = 15360.0 - 59.0


def build_nc(seq=N, qd=QDIM, nh=HEADS_PER_CORE, hd=HEAD_DIM, dout=QDIM,
             scale=SCALE, pool_jts=(), dve_jts=(3, 5, 8, 10, 13, 15),
             dve_jts_late=None, late_from=8,
             pool_skip_first=1, last_norm='dve', norm_mode='dve',
             gap_ns=390, expp_bufs=8, y16=True, ihw=512, h_outer=False,
             norm_eng='pool', pend_jt=4, ts_eng='dve', kq_act=1,
             first_split=False, last_split=False, kq1_after=7, up_act=0,
             tail_act=False, last_dve_jts=None,
             budget_f=1.15,
             sim_bufs=4, uacc_bufs=4, dbg=False, eager=False):
    """Build the per-core Bass program (same program on all 8 cores)."""
    from contextlib import ExitStack

    import concourse.bass as bass
    import concourse.tile as tile
    from concourse import bacc, mybir

    P = 128
    f16 = mybir.dt.float16
    f32 = mybir.dt.float32
    i16 = mybir.dt.int16
    i32 = mybir.dt.int32
    Exp = mybir.ActivationFunctionType.Exp
    Ln = mybir.ActivationFunctionType.Ln
    Mul = mybir.AluOpType.mult
    Add = mybir.AluOpType.add
    Xor = mybir.AluOpType.bitwise_xor

    dmc = nh * hd                  # 256
    kt = qd // P                   # 8 contraction tiles
    seqt = seq // P                # 16 j-tiles
    mtiles = dmc // P              # 2 qT/kT partition tiles
    npairs = mtiles
    n_ih = seq // ihw              # 2 i-halves per head
    nsub = ihw // P                # 8 i-subtiles per phase
    ncw = seq // 512               # 4 column chunks (proj granule width 512)
    ydt = f16 if y16 else f32

    nc = bacc.Bacc("TRN2", target_bir_lowering=False, debug=False,
                   num_devices=N_CORES)

    xt = nc.dram_tensor("xt", (qd, seq), f16, kind="ExternalInput").ap()
    wq = nc.dram_tensor("wq", (qd, dmc), f16, kind="ExternalInput").ap()
    wk = nc.dram_tensor("wk", (qd, dmc), f16, kind="ExternalInput").ap()
    wv = nc.dram_tensor("wv", (qd, dmc), f16, kind="ExternalInput").ap()
    wo = nc.dram_tensor("wo", (dmc, dout), f16, kind="ExternalInput").ap()
    ident = nc.dram_tensor("ident", (P, P), f16, kind="ExternalInput").ap()
    y = nc.dram_tensor("y", (seq, dout), ydt, kind="ExternalOutput").ap()
    dbgu = (nc.dram_tensor("dbgu", (P, npairs, seq), f16,
                           kind="ExternalOutput").ap() if dbg else None)
    dbgq = (nc.dram_tensor("dbgq", (P, mtiles, seq), f16,
                           kind="ExternalOutput").ap() if dbg else None)
    dbgk = (nc.dram_tensor("dbgk", (P, mtiles, seq), f16,
                           kind="ExternalOutput").ap() if dbg else None)
    dbgv = (nc.dram_tensor("dbgv", (P, seqt, nh, hd + 1), f16,
                           kind="ExternalOutput").ap() if dbg else None)
    dbgs = (nc.dram_tensor("dbgs", (P, 512), f32,
                           kind="ExternalOutput").ap() if dbg else None)
    dbga = (nc.dram_tensor("dbga", (P, 4, P), f32,
                           kind="ExternalOutput").ap() if dbg else None)
    dbge = (nc.dram_tensor("dbge", (P, 512), f16,
                           kind="ExternalOutput").ap() if dbg else None)

    with tile.TileContext(nc) as tc, ExitStack() as ctx:
        # ---- SBUF pools ----
        persist = ctx.enter_context(tc.tile_pool(name="persist", bufs=1))
        expp = ctx.enter_context(tc.tile_pool(name="expp", bufs=expp_bufs))
        u16p = ctx.enter_context(tc.tile_pool(name="u16p", bufs=2))
        rws = ctx.enter_context(tc.tile_pool(name="rws", bufs=2))
        ysb = ctx.enter_context(tc.tile_pool(name="ysb", bufs=3))
        # ---- PSUM pools (8 banks: 4 sim + 2 uacc + 2 spare) ----
        simp = ctx.enter_context(tc.tile_pool(name="simp", bufs=sim_bufs,
                                              space="PSUM"))
        uaccp = ctx.enter_context(tc.tile_pool(name="uaccp", bufs=uacc_bufs,
                                               space="PSUM"))
        spare = simp

        # ---- persistent SBUF tensors ----
        xt_sb = persist.tile([P, kt, seq], f16)
        wq_sb = persist.tile([P, kt, dmc], f16)
        wk_sb = persist.tile([P, kt, dmc], f16)
        wv_sb = persist.tile([P, kt, dmc], f16)
        wo_sb = persist.tile([P, npairs, dout], f16)
        v_sb = persist.tile([P, seqt, nh, hd + 1], f16)
        qt_sb = persist.tile([P, mtiles, seq], f16)
        kt_sb = persist.tile([P, mtiles, seq], f16)
        ident_sb = persist.tile([P, P], f16)
        upairs = [persist.tile([P, seq], f16, name=f"upair{p}")
                  for p in range(npairs)]

        # ---- input DMAs, ordered for earliest compute start ----
        def dma_w(dst, src, lo, hi):
            nc.sync.dma_start(
                dst[:, :, lo:hi],
                src[:, lo:hi].rearrange("(ko ki) m -> ki ko m", ki=P))

        def dma_xt(c):
            nc.sync.dma_start(
                xt_sb[:, :, c * 512:(c + 1) * 512],
                xt[:, c * 512:(c + 1) * 512].rearrange(
                    "(ko ki) s -> ki ko s", ki=P))

        dma_w(wk_sb, wk, 0, P)
        nc.sync.dma_start(
            xt_sb[:, :, 0:256],
            xt[:, 0:256].rearrange("(ko ki) s -> ki ko s", ki=P))
        dma_w(wq_sb, wq, 0, P)
        nc.sync.dma_start(
            xt_sb[:, :, 256:512],
            xt[:, 256:512].rearrange("(ko ki) s -> ki ko s", ki=P))
        dma_xt(1)
        dma_w(wv_sb, wv, 0, dmc)
        dma_xt(2)
        dma_xt(3)
        dma_w(wk_sb, wk, P, dmc)
        dma_w(wq_sb, wq, P, dmc)
        nc.sync.dma_start(wo_sb[:],
                          wo.rearrange("(t p) n -> p t n", p=P))
        nc.sync.dma_start(ident_sb[:], ident[:])
        nc.vector.memset(v_sb[:, :, :, hd], 1.0)

        # ================= micro-task filler =================
        class Gran:
            def __init__(self, key, micros):
                self.key = key
                self.micros = micros  # list of (ns, fn)
                self.i = 0

            def pending(self):
                return self.i < len(self.micros)

            def step(self):
                ns, fn = self.micros[self.i]
                self.i += 1
                fn()
                return ns

        grans = {}
        order = []
        pending_ns = [0.0]

        def add_gran(key, micros):
            g = Gran(key, micros)
            grans[key] = g
            order.append(g)
            pending_ns[0] += sum(ns for ns, _ in micros)

        def force(key):
            g = grans[key]
            while g.pending():
                pending_ns[0] -= g.micros[g.i][0]
                g.step()

        def fill(budget):
            for g in order:
                while g.pending() and budget > 100:
                    ns = g.micros[g.i][0]
                    pending_ns[0] -= ns
                    budget -= ns
                    g.step()
                if budget <= 100:
                    break

        # ---- projection granules ----
        def kq_gran(which, mt, c, n0=None, nw=512):
            """kT (which=0) / qT (which=1) tile [mt*128, n0:n0+nw]."""
            w_sb, out_sb = ((wk_sb, kt_sb), (wq_sb, qt_sb))[which]
            if n0 is None:
                n0 = c * 512
            cell = {}

            def mm(ko):
                def f():
                    if ko == 0:
                        cell["ps"] = spare.tile([P, nw], f32, tag="sim",
                                                name="ps",
                                                padded_shape=[P, 512])
                    for k in (ko, ko + 1):
                        nc.tensor.matmul(
                            cell["ps"][:],
                            lhsT=w_sb[:, k, mt * P:(mt + 1) * P],
                            rhs=xt_sb[:, k, n0:n0 + nw],
                            start=(k == 0), stop=(k == kt - 1))
                    if ko == kt - 2:
                        if kq_act:
                            nc.scalar.activation(
                                out_sb[:, mt, n0:n0 + nw],
                                cell["ps"][:],
                                mybir.ActivationFunctionType.Copy)
                        else:
                            nc.vector.tensor_copy(
                                out_sb[:, mt, n0:n0 + nw],
                                cell["ps"][:])
                return f

            return [(440 * nw // 512 + 30, mm(ko)) for ko in range(0, kt, 2)]

        def v_gran(jt):
            """v_sb[:, jt, :, 0:hd] (2 micros)."""
            cell = {}

            def mm(ko):
                def f():
                    if ko == 0:
                        cell["ps"] = spare.tile([P, dmc], f32, tag="sim",
                                                name="psv")
                    for k in range(ko, ko + 4):
                        nc.tensor.matmul(
                            cell["ps"][:],
                            lhsT=xt_sb[:, k, jt * P:(jt + 1) * P],
                            rhs=wv_sb[:, k, :],
                            start=(k == 0), stop=(k == kt - 1))
                    if ko == kt - 4:
                        nc.vector.tensor_copy(
                            v_sb[:, jt, :, 0:hd],
                            cell["ps"].rearrange("p (h d) -> p h d", h=nh))
                return f

            return [(440, mm(0)), (440, mm(4))]

        def f_gran2(m, act_copy=False):
            """final proj for rows m*128..(m+2)*128: one DMA (6 micros)."""
            cell = {}
            Copy = mybir.ActivationFunctionType.Copy

            def ycopy(dst, src, on_act):
                if on_act:
                    nc.scalar.activation(dst, src, Copy)
                else:
                    nc.vector.tensor_copy(dst, src)

            def mm(t, ci):
                def f():
                    if t == 0 and ci == 0:
                        cell["yt"] = ysb.tile([P, 2, dout], ydt, tag="yt2",
                                              name="yt")
                    if not (t == 0 and ci == 0):
                        pt, pc = (t, ci - 1) if ci else (t - 1, 1)
                        ycopy(cell["yt"][:, pt, pc * 512:(pc + 1) * 512],
                              cell["ps"], (ci == 1) != act_copy)
                    ps = spare.tile([P, 512], f32, tag="sim", name="yps")
                    cell["ps"] = ps
                    for p in range(npairs):
                        nc.tensor.matmul(
                            ps[:],
                            lhsT=upairs[p][:, (m + t) * P:(m + t + 1) * P],
                            rhs=wo_sb[:, p, ci * 512:(ci + 1) * 512],
                            start=(p == 0), stop=(p == npairs - 1))
                return f

            def out():
                ycopy(cell["yt"][:, 1, 512:1024], cell["ps"], not act_copy)
                nc.sync.dma_start(
                    y[m * P:(m + 2) * P, :].rearrange(
                        "(t p) n -> p t n", p=P),
                    cell["yt"][:])

            return ([(470, mm(t, ci)) for t in (0, 1) for ci in (0, 1)]
                    + [(80, out)])

        def f_gran(m, act_copy=False):
            """final proj y[m*128:(m+1)*128, :] (3 micros)."""
            cell = {}
            Copy = mybir.ActivationFunctionType.Copy

            def ycopy(dst, src, on_act):
                if on_act:
                    nc.scalar.activation(dst, src, Copy)
                else:
                    nc.vector.tensor_copy(dst, src)

            def mm(ci):
                def f():
                    if ci == 0:
                        cell["yt"] = ysb.tile([P, dout], ydt, tag="yt",
                                              name="yt")
                    else:
                        ycopy(cell["yt"][:, 0:512], cell["ps0"][:], True)
                        if not act_copy:
                            nc.sync.dma_start(y[m * P:(m + 1) * P, 0:512],
                                              cell["yt"][:, 0:512])
                    ps = spare.tile([P, 512], f32, tag="sim", name="yps")
                    cell[f"ps{ci}"] = ps
                    for p in range(npairs):
                        nc.tensor.matmul(
                            ps[:],
                            lhsT=upairs[p][:, m * P:(m + 1) * P],
                            rhs=wo_sb[:, p, ci * 512:(ci + 1) * 512],
                            start=(p == 0), stop=(p == npairs - 1))
                return f

            def out():
                ycopy(cell["yt"][:, 512:1024], cell["ps1"][:], False)
                if act_copy:
                    nc.sync.dma_start(y[m * P:(m + 1) * P, :], cell["yt"][:])
                else:
                    nc.sync.dma_start(y[m * P:(m + 1) * P, 512:1024],
                                      cell["yt"][:, 512:1024])

            return [(440, mm(0)), (440, mm(1)), (80, out)]

        # deadline-ordered granule queue (finals appended as they unlock)
        for c in range(ncw):
            if c == 0:
                halves = (kq_gran(0, 0, 0, 0, 256)
                          + kq_gran(0, 0, 0, 256, 256))
                add_gran(("k", 0, 0), halves)
                add_gran(("q", 0, 0), (kq_gran(1, 0, 0, 0, 256)
                                       + kq_gran(1, 0, 0, 256, 256)))
                for cq in range(1, ncw):
                    add_gran(("q", 0, cq), kq_gran(1, 0, cq))
            else:
                add_gran(("k", 0, c), kq_gran(0, 0, c))
        for jt in range(seqt):
            add_gran(("v", jt), v_gran(jt))
            if jt == min(kq1_after, seqt - 1):
                add_gran(("k", 1, 0), kq_gran(0, 1, 0))
                add_gran(("q", 1, 0), kq_gran(1, 1, 0))
        for c in range(1, ncw):
            add_gran(("k", 1, c), kq_gran(0, 1, c))
        for c in range(1, ncw):
            add_gran(("q", 1, c), kq_gran(1, 1, c))

        if eager:
            for g in list(order):
                while g.pending():
                    pending_ns[0] -= g.micros[g.i][0]
                    g.step()

        # ================= attention =================
        phases = []
        for q in range(n_ih):
            for h in range(nh):
                i0, iw = q * ihw, ihw
                if q == 0 and h == 0 and first_split:
                    phases += [(h, i0, iw // 2), (h, i0 + iw // 2, iw // 2)]
                elif q == n_ih - 1 and h == nh - 1 and last_split:
                    phases += [(h, i0, iw // 2), (h, i0 + iw // 2, iw // 2)]
                else:
                    phases.append((h, i0, iw))
        # deferred per-phase PE tail (transposes), run early in the NEXT phase
        pending_pe = [None]

        def attn_phase(idx, h, i0, iw):
            mt, hb, pair = h // 2, (h % 2) * hd, h // 2
            nsub = iw // P
            is_last = idx == len(phases) - 1
            nmode = last_norm if is_last else norm_mode
            slots_left = (len(phases) - idx) * seqt
            slot_budget = min(900.0, budget_f * pending_ns[0] / slots_left)
            uacc = [uaccp.tile([P, hd + 1], f32, tag="uacc",
                               name=f"ua{idx}_{s}") for s in range(nsub)]
            sims, expts = {}, {}

            def emit_sim(jt):
                force(("k", mt, (jt * P) // 512))
                for c in range(i0 // 512, (i0 + iw + 511) // 512):
                    force(("q", mt, c))
                sim = simp.tile([P, iw], f32, tag="sim", padded_shape=[P, 512])
                for c0 in range(0, iw, 512):
                    cw = min(512, iw - c0)
                    nc.tensor.matmul(
                        sim[:, c0:c0 + cw],
                        lhsT=kt_sb[hb:hb + hd, mt, jt * P:(jt + 1) * P],
                        rhs=qt_sb[hb:hb + hd, mt, i0 + c0:i0 + c0 + cw],
                        start=True, stop=True)
                sims[jt] = sim

            def emit_exp(jt):
                expt = expp.tile([P, iw], f16, tag="expt",
                                 padded_shape=[P, 512])
                djts = (dve_jts_late if (dve_jts_late is not None
                                         and idx >= late_from) else dve_jts)
                if is_last and last_dve_jts is not None:
                    djts = last_dve_jts
                sch = idx >= pool_skip_first and (jt in pool_jts
                                                  or jt in djts)
                if sch:
                    nc.vector.tensor_scalar(
                        expt[:].bitcast(i16), sims[jt][:],
                        float(SCH_A), float(SCH_B), op0=Mul, op1=Add)
                else:
                    nc.scalar.activation(expt[:], sims[jt][:], Exp,
                                         scale=scale)
                expts[jt] = expt

            def emit_av(jt):
                force(("v", jt))
                for s in range(nsub):
                    nc.tensor.matmul(
                        uacc[s][:, 0:hd + 1],
                        lhsT=expts[jt][:, s * P:(s + 1) * P],
                        rhs=v_sb[:, jt, h, :],
                        start=(jt == 0), stop=(jt == seqt - 1))

            depth = sim_bufs - 1
            for j0 in range(depth):
                emit_sim(j0)
                emit_exp(j0)
            for jt in range(seqt + 1):
                fill(slot_budget)
                if jt == pend_jt and pending_pe[0] is not None:
                    pending_pe[0]()
                    pending_pe[0] = None
                if jt + depth < seqt:
                    emit_sim(jt + depth)
                    emit_exp(jt + depth)
                if jt >= 1:
                    emit_av(jt - 1)

            # norm (DVE work, emitted now); transposes deferred to next phase
            u16 = u16p.tile([P, nsub, hd], f16, tag="u16",
                            padded_shape=[P, ihw // P, hd])
            srow = rws.tile([P, nsub], f32, tag="srow", name="srow")
            for s in range(nsub):
                nc.vector.tensor_copy(srow[:, s:s + 1], uacc[s][:, hd:hd + 1])
            rrow = rws.tile([P, nsub], f32, tag="rrow", name="rrow")
            neg = nmode != "ln"
            if nmode == "ln":
                lnrow = rws.tile([P, nsub], f32, tag="lnrow", name="lnrow")
                nc.scalar.activation(lnrow[:], srow[:], Ln)
                nc.scalar.activation(rrow[:], lnrow[:], Exp, scale=-1.0)
            else:
                # Newton bit-trick: u ~= -1/s, two iterations, then negate.
                trow = rws.tile([P, nsub], f32, tag="trow", name="trow")
                if norm_eng == 'pool' and not is_last:
                    ne = nc.gpsimd
                    nc.vector.tensor_scalar(trow[:].bitcast(i32),
                                            srow[:].bitcast(i32), -1, None,
                                            op0=Xor)
                    ne.tensor_scalar_mul(rrow[:], trow[:], 0.23549792)
                    for cst in (2.0017324, 2.0):
                        ne.tensor_mul(trow[:], srow[:], rrow[:])
                        ne.tensor_scalar(trow[:], trow[:], float(cst), None,
                                         op0=Add)
                        ne.tensor_mul(rrow[:], trow[:], rrow[:])
                    ne.tensor_scalar_mul(rrow[:], rrow[:], -1.0)
                else:
                    ne = nc.vector
                    ne.tensor_scalar(trow[:].bitcast(i32),
                                     srow[:].bitcast(i32), -1, None, op0=Xor)
                    ne.tensor_scalar_mul(rrow[:], trow[:], 0.23549792)
                    for cst in (2.0017324, 2.0):
                        ne.tensor_mul(trow[:], srow[:], rrow[:])
                        ne.scalar_tensor_tensor(rrow[:], trow[:], float(cst),
                                                rrow[:], op0=Add, op1=Mul)
                    ne.tensor_scalar_mul(rrow[:], rrow[:], -1.0)
            for s in range(nsub):
                if ts_eng == 'act' and not is_last:
                    nc.scalar.activation(
                        u16[:, s, :], uacc[s][:, 0:hd],
                        mybir.ActivationFunctionType.Copy,
                        scale=rrow[:, s:s + 1])
                else:
                    nc.vector.tensor_scalar(
                        u16[:, s, :], uacc[s][:, 0:hd],
                        rrow[:, s:s + 1], None, op0=Mul)

            def tail():
                for sp in range(nsub // 2):
                    tp = spare.tile([hd, 2 * P], f16, tag="sim", name="tp")
                    nc.tensor.transpose(tp[:, 0:P], u16[:, 2 * sp, :],
                                        ident_sb[:])
                    nc.tensor.transpose(tp[:, P:2 * P], u16[:, 2 * sp + 1, :],
                                        ident_sb[:])
                    udst = upairs[pair][hb:hb + hd,
                                        i0 + sp * 2 * P:i0 + (sp + 1) * 2 * P]
                    if is_last or (up_act and sp % 2 == 0):
                        nc.scalar.activation(
                            udst, tp[:], mybir.ActivationFunctionType.Copy)
                    else:
                        nc.vector.tensor_copy(udst, tp[:])
                if h == nh - 1:
                    for m in range(i0 // P, (i0 + iw) // P, 2):
                        add_gran(("f", m),
                                 f_gran2(m, act_copy=(tail_act
                                                      and i0 // ihw
                                                      == n_ih - 1)))

            pending_pe[0] = tail

        for idx, (h, i0, iw) in enumerate(phases):
            attn_phase(idx, h, i0, iw)
        pending_pe[0]()
        for g in order:
            while g.pending():
                g.step()
        if dbg:
            for p in range(npairs):
                nc.sync.dma_start(dbgu[:, p, :], upairs[p][:])
            nc.sync.dma_start(dbgq[:], qt_sb[:])
            nc.sync.dma_start(dbgk[:], kt_sb[:])
            nc.sync.dma_start(dbgv[:], v_sb[:])

    nc.compile()
    return nc


_NC_CACHE = {}


def _get_nc():
    if "nc" not in _NC_CACHE:
        _NC_CACHE["nc"] = build_nc()
    return _NC_CACHE["nc"]


def _prep_core_inputs(x, Wq, Wkv, Wo):
    """Host-side shard + layout prep: per-core fp16 slices."""
    f16 = np.float16
    eye = np.eye(128, dtype=f16)
    in_maps = []
    for c in range(N_CORES):
        b, g = c // 2, c % 2
        s = slice(g * DMC, (g + 1) * DMC)
        in_maps.append({
            "xt": np.ascontiguousarray(x[b].T).astype(f16),
            "wq": np.ascontiguousarray(Wq[:, s]).astype(f16),
            "wk": np.ascontiguousarray(Wkv[:, g * DMC:(g + 1) * DMC]).astype(f16),
            "wv": np.ascontiguousarray(
                Wkv[:, DIM_MODEL + g * DMC:DIM_MODEL + (g + 1) * DMC]).astype(f16),
            "wo": np.ascontiguousarray(Wo[s, :]).astype(f16),
            "ident": eye,
        })
    return in_maps


def kernel(x, Wq, Wkv, Wo, bo):
    from concourse import bass_utils

    x = np.asarray(x, dtype=np.float32)
    Wq = np.asarray(Wq, dtype=np.float32)
    Wkv = np.asarray(Wkv, dtype=np.float32)
    Wo = np.asarray(Wo, dtype=np.float32)
    bo = np.asarray(bo, dtype=np.float32)

    nc = _get_nc()
    in_maps = _prep_core_inputs(x, Wq, Wkv, Wo)
    res = bass_utils.run_bass_kernel_spmd(nc, in_maps,
                                          core_ids=list(range(N_CORES)))
    out = np.empty((B, N, QDIM), dtype=np.float32)
    for b in range(B):
        out[b] = (res.results[2 * b]["y"].astype(np.float32)
                  + res.results[2 * b + 1]["y"].astype(np.float32) + bo)
    return out
